# revision 110
# baseline (speedup 1.0000x reference)
# Multi-head attention kernel for Trainium2, sharded over 8 NeuronCores.
#
# Sharding: core = (batch b, query-half qh, head-half hh). Each core handles
# 1024 queries of one batch for 6 of the 12 heads, computing k/v projections
# for its 6 heads over the full 2048 keys (2x redundancy across query-halves
# only; cross-core collectives are slower than recompute on this chip). The
# output projection is a partial sum over the core's 384 head-dims; the two
# head-half partners are summed on the host in assemble().
#
# Layout strategy (bf16 matmul operands, fp32 PSUM accumulation/epilogues):
#   - Host pre-transposes activations to [E, S] so the contraction dim (E)
#     lands on SBUF partitions; all matmul operands are bf16 (fp32 matmul
#     lowers to two PE passes).
#   - q^T, k^T computed as [384, S] via lhsT=W chunks; per-partition bias
#     added during the PSUM->SBUF copy.
#   - v computed directly as [keys, 384] with a ones-column per head
#     ([128,16,6,65]) so the PV matmul (M=65) also produces the softmax
#     denominator row for free.
#   - scores^T = [keys, queries] per head: K=64 matmuls; even/odd heads of a
#     pair sit in partition halves 0-63/64-127, emitted adjacently so they
#     land in disjoint PE row groups and stream concurrently (row packing).
#   - exp on ScalarE in [128, 2x512] groups PSUM->SBUF (bf16), streamed
#     straight into the accumulating PV matmul.
#   - softmax normalization runs one iteration behind the attention stream
#     (fp16 K=2 broadcast matmul + DVE multiplies), so the in-order engine
#     queues never stall on its dependency chain.
#   - output projection contracts head pairs as K=128 matmuls; epilogue adds
#     the host-precomputed partial bias (bv_half@Wo_half + bo/2).

import numpy as np
from contextlib import ExitStack

import concourse.bass as bass
import concourse.mybir as mybir
import concourse.tile as tile
from concourse import bacc
from concourse.bass_utils import run_bass_kernel_spmd

F32 = mybir.dt.float32
BF16 = mybir.dt.bfloat16
F16 = mybir.dt.float16
P = 128
E = 768
S = 2048
B = 2
H = 12            # total heads
HC = 6            # heads per core
D = 64
EH = HC * D       # 384 head-dims per core
QB = 1024         # queries per core
NCORES = 8
EC = E // P       # 6 e-chunks (contraction tiles)
KT = S // P       # 16 key tiles
MT = EH // P      # 3 M-tiles for q^T/k^T (384 rows)
NC4 = S // 512    # 4 n-slices of k^T
NOUT = 6          # attention outer iterations: (pair, query-slice)


def build_nc():
    nc = bacc.Bacc("TRN2", debug=False)

    # DRAM I/O (per-core shapes; same NEFF on all 8 cores)
    xq = nc.dram_tensor("xq", (E, QB), BF16, kind="ExternalInput")     # query[b,half].T
    xk = nc.dram_tensor("xk", (E, S), BF16, kind="ExternalInput")      # key[b].T
    xv = nc.dram_tensor("xv", (E, S), BF16, kind="ExternalInput")      # value[b].T
    wq = nc.dram_tensor("wq", (E, EH), BF16, kind="ExternalInput")     # head-half slice, pre-scaled
    wk = nc.dram_tensor("wk", (E, EH), BF16, kind="ExternalInput")
    wv = nc.dram_tensor("wv", (E, EH), BF16, kind="ExternalInput")
    wo = nc.dram_tensor("wo", (EH, E), BF16, kind="ExternalInput")
    bq = nc.dram_tensor("bq", (P, MT), F32, kind="ExternalInput")      # per-partition bias per M-tile
    bk = nc.dram_tensor("bk", (P, MT), F32, kind="ExternalInput")
    bo = nc.dram_tensor("bo", (P, E), F32, kind="ExternalInput")       # bv_half@Wo_half + bo/2
    seld = nc.dram_tensor("seld", (2, P), F16, kind="ExternalInput")   # pair-broadcast selector
    out = nc.dram_tensor("out", (QB, E), F32, kind="ExternalOutput")   # partial (head-half) output

    with tile.TileContext(nc) as tc:
        with ExitStack() as ctx:
            _emit(ctx, tc, nc, xq, xk, xv, wq, wk, wv, wo, bq, bk, bo, seld, out)
    nc.compile()
    return nc


def _emit(ctx, tc, nc, xq, xk, xv, wq, wk, wv, wo, bq, bk, bo, seld, out):
    # ---- pools ----
    persist = ctx.enter_context(tc.tile_pool(name="persist", bufs=1))
    wpool = ctx.enter_context(tc.tile_pool(name="wpool", bufs=2))
    xpool = ctx.enter_context(tc.tile_pool(name="xpool", bufs=2))
    epool = ctx.enter_context(tc.tile_pool(name="epool", bufs=4))
    spool = ctx.enter_context(tc.tile_pool(name="spool", bufs=2))
    outpool = ctx.enter_context(tc.tile_pool(name="outpool", bufs=2))
    # PSUM pools (8 banks total): scores/oproj 4 + PV accumulators 2 +
    # vproj/norm-bc 1 + projection chunks 1
    psA = ctx.enter_context(tc.tile_pool(name="psA", bufs=2, space="PSUM"))
    psB = ctx.enter_context(tc.tile_pool(name="psB", bufs=1, space="PSUM"))
    psC = ctx.enter_context(tc.tile_pool(name="psC", bufs=2, space="PSUM"))
    psD = ctx.enter_context(tc.tile_pool(name="psD", bufs=1, space="PSUM"))

    # ---- persistent SBUF tensors ----
    qT = persist.tile([P, MT, QB], BF16)          # q^T [384, 1024]
    kT = persist.tile([P, MT, S], BF16)           # k^T [384, 2048]
    # key^T/value^T staged per 512-key slice: separate tiles keep dependency
    # tracking slice-granular (one big tile makes every reader wait for the
    # last slice DMA)
    xv_sb = [persist.tile([P, EC, 512], BF16, tag=f"xv{n}", name=f"xv{n}")
             for n in range(NC4)]
    xk_sb = [persist.tile([P, EC, 512], BF16, tag=f"xk{n}", name=f"xk{n}")
             for n in range(NC4)]
    v_sb = persist.tile([P, KT, HC, D + 1], BF16)  # v + ones column per head
    o_all = persist.tile([P, MT, QB], BF16)       # normalized o^T, pair-packed
    bq_sb = persist.tile([P, MT], F32)
    bk_sb = persist.tile([P, MT], F32)
    bo_sb = persist.tile([P, E], F32)
    o_raw = persist.tile([D + 1, HC, 2, 512], F32)  # unnormalized o^T per (head, qs)
    sel2 = persist.tile([2, P], F16)              # pair broadcast selector
    dens = [persist.tile([2, 512], F32, tag=f"dens{it}", name=f"dens{it}")
            for it in range(NOUT)]
    drec = [persist.tile([2, 512], F16, tag=f"drec{it}", name=f"drec{it}")
            for it in range(NOUT)]

    # first-needed DMAs first; constants go on the scalar HWDGE queue
    wq_t = wpool.tile([P, EC, EH], BF16, tag="w18")
    xq_t = xpool.tile([P, EC, QB], BF16, tag="xq")
    for ec in range(EC):
        nc.sync.dma_start(wq_t[:, ec, :], wq[ec * P:(ec + 1) * P, :])
        nc.sync.dma_start(xq_t[:, ec, :], xq[ec * P:(ec + 1) * P, :])
    nc.scalar.dma_start(bq_sb[:], bq[:])
    nc.scalar.dma_start(bk_sb[:], bk[:])
    nc.scalar.dma_start(bo_sb[:], bo[:])
    nc.scalar.dma_start(sel2[:], seld[:])

    # ones columns for denominator (written once; v-proj copies don't touch col D)
    nc.vector.memset(v_sb[:, :, :, D], 1.0)

    # ---- projection chunk emitters ----
    # The projections are consumed M-tile-wise: kT/qT tile mt and v head-pair
    # p feed only attention iterations 2mt/2mt+1 (resp. 2p/2p+1). Each chunk
    # is emitted just-in-time inside the iteration stream, so projection work
    # fills the PE slack under the exp pipeline instead of serializing ahead
    # of it. Chunk PSUM lives in its own 1-bank pool (psD) so the ring never
    # collides with the live PV accumulators.
    def _qproj(mt, n2):
        ps = psD.tile([P, 512], F32, tag="psD", name="qp")
        for ec in range(EC):
            nc.tensor.matmul(ps[:], wq_t[:, ec, mt * P:(mt + 1) * P],
                             xq_t[:, ec, n2 * 512:(n2 + 1) * 512],
                             start=(ec == 0), stop=(ec == EC - 1))
        nc.vector.tensor_scalar_add(qT[:, mt, n2 * 512:(n2 + 1) * 512], ps[:],
                                    bq_sb[:, mt:mt + 1])

    def _kproj(mt, n4):
        ps = psD.tile([P, 512], F32, tag="psD", name="kp")
        for ec in range(EC):
            nc.tensor.matmul(ps[:], wk_t[:, ec, mt * P:(mt + 1) * P],
                             xk_sb[n4][:, ec, :],
                             start=(ec == 0), stop=(ec == EC - 1))
        nc.vector.tensor_scalar_add(kT[:, mt, n4 * 512:(n4 + 1) * 512], ps[:],
                                    bk_sb[:, mt:mt + 1])

    def _vproj(p, kt):
        psv = psB.tile([P, 512], F32, tag="psB", name="psv")
        off = (kt % 4) * P
        for ec in range(EC):
            nc.tensor.matmul(psv[:, 0:P], xv_sb[kt // 4][:, ec, off:off + P],
                             wv_t[:, ec, p * P:(p + 1) * P],
                             start=(ec == 0), stop=(ec == EC - 1))
        # strided copy into per-head slots (leaves ones column intact)
        nc.vector.tensor_copy(v_sb[:, kt, 2 * p:2 * p + 2, 0:D],
                              psv[:, 0:P].rearrange("p (h d) -> p h d", d=D))

    # ---- attention helpers ----
    # Per key tile: both heads' score matmuls are adjacent K=64 ops on
    # disjoint PE row groups (partitions 0-63 / 64-127) -> run concurrently.
    # Softmax normalization is pipelined one iteration behind the attention
    # stream so the in-order PE/DVE queues never stall on its chain.
    def _attn_scores(hp, qs, kt):
        st = psC.tile([P, 2, 512], F32, tag="psC", name="st")
        for i in range(2):
            po = D * i      # partition offset of this head's d-rows
            nc.tensor.matmul(st[:, i, :],
                             kT[po:po + D, hp, kt * P:(kt + 1) * P],
                             qT[po:po + D, hp, qs * 512:(qs + 1) * 512],
                             start=True, stop=True)
        ex = epool.tile([P, 2, 512], BF16, tag="ex")
        nc.scalar.activation(ex[:, :, :], st[:, :, :], mybir.ActivationFunctionType.Exp)
        return ex

    def _attn_pv(o_ps, hp, ex, kt):
        for i in range(2):
            nc.tensor.matmul(o_ps[i][0:D + 1, :],
                             v_sb[:, kt, 2 * hp + i, :],
                             ex[:, i, :],
                             start=(kt == 0), stop=(kt == KT - 1))

    def _drain(o_ps, it):
        # per-head copy releases the PSUM bank (incl. denom row); the
        # denominator pair is DMA-gathered onto partitions 0-1 so ONE
        # partition-parallel DVE reciprocal covers both heads.
        hp, qs = divmod(it, 2)
        for i in range(2):
            nc.vector.tensor_copy(o_raw[:, 2 * hp + i, qs, :], o_ps[i][0:D + 1, :])
            nc.sync.dma_start(dens[it][i:i + 1, :], o_raw[D:D + 1, 2 * hp + i, qs, :])
        with nc.allow_low_precision(reason="fp16 reciprocal feeds fp16 broadcast matmul; den ~1e3, ample range"):
            nc.vector.reciprocal(drec[it][:], dens[it][:])

    # ---- stage k^T/v^T activations and weights; slice 0 ahead of the bulk
    # so the first kproj/vproj chunks are not DMA-gated ----
    wk_t = wpool.tile([P, EC, EH], BF16, tag="w18")
    for ec in range(EC):
        nc.sync.dma_start(wk_t[:, ec, :], wk[ec * P:(ec + 1) * P, :])
    nc.sync.dma_start(xk_sb[0][:], xk[:, 0:512].rearrange("(ec p) s -> p ec s", p=P))
    nc.sync.dma_start(xv_sb[0][:], xv[:, 0:512].rearrange("(ec p) s -> p ec s", p=P))
    wv_t = wpool.tile([P, EC, EH], BF16, tag="wv")
    for ec in range(EC):
        nc.sync.dma_start(wv_t[:, ec, :], wv[ec * P:(ec + 1) * P, :])
    for n4 in range(1, NC4):
        nc.sync.dma_start(xk_sb[n4][:],
                          xk[:, n4 * 512:(n4 + 1) * 512].rearrange("(ec p) s -> p ec s", p=P))
        nc.sync.dma_start(xv_sb[n4][:],
                          xv[:, n4 * 512:(n4 + 1) * 512].rearrange("(ec p) s -> p ec s", p=P))
    _qproj(0, 0)
    _kproj(0, 0)

    def _norm_bc(it):
        # one fp16 K=2 matmul broadcasts both reciprocals of the pair into
        # partition halves of the (idle during attention) psB bank
        bc = psB.tile([P, 512], F32, tag="psB", name="bc")
        nc.tensor.matmul(bc[:], sel2[:], drec[it][:], start=True, stop=True)
        return bc

    def _norm_tt(it, bc):
        hp, qs = divmod(it, 2)
        for i in range(2):
            nc.vector.tensor_tensor(o_all[D * i:D * (i + 1), hp, qs * 512:(qs + 1) * 512],
                                    o_raw[0:D, 2 * hp + i, qs, :],
                                    bc[D * i:D * (i + 1), :], mybir.AluOpType.mult)

    bc_prev = None
    for it in range(NOUT):
        hp, qs = divmod(it, 2)
        o_ps = {}
        for i in range(2):
            o_ps[i] = psA.tile([P, 512], F32, tag="psA", name=f"o_ps{i}")
        for kt in range(KT):
            # scores + exp go first in the slot so the ACT pipeline is fed as
            # early as possible; the projection fillers then run in the PE
            # window while exp computes, and the PVs close the slot.
            ex = _attn_scores(hp, qs, kt)
            if it % 2 == 0:
                # even iterations first consume pair hp: its v chunks emit
                # just-in-time, and kT tile hp's remaining key-slices emit
                # two slots ahead of their first consuming scores. Pairs > 0
                # had their first 8 v chunks emitted in the previous (lighter)
                # odd iteration.
                if hp == 0 or kt >= 8:
                    _vproj(hp, kt)
                if kt in (2, 6, 10):
                    _kproj(hp, kt // 4 + 1)
            else:
                if hp + 1 < MT and kt % 2 == 0:
                    _vproj(hp + 1, kt // 2)
                if kt == 12 and hp + 1 < MT:
                    _kproj(hp + 1, 0)
            if kt == 8 and it + 1 < NOUT:
                nmt, nn2 = divmod(it + 1, 2)
                _qproj(nmt, nn2)
            if it > 0:
                if kt == 8:
                    bc_prev = _norm_bc(it - 1)
                if kt == 12:
                    _norm_tt(it - 1, bc_prev)
            _attn_pv(o_ps, hp, ex, kt)
        _drain(o_ps, it)
    # ---- output projection (partial: this core's 384 head-dims) ----
    # Double-buffered through the psC pool (scores are done with it). The
    # first half (query tiles 0-3) depends only on qs=0 normalizations, which
    # completed before the last attention iteration — emitting it before the
    # final norm keeps the PE busy (and its clock warm) through that chain.
    wo_t = wpool.tile([P, MT, E], BF16, tag="w18")
    nc.sync.dma_start(wo_t[:], wo[:].rearrange("(mt p) m -> p mt m", p=P))

    def _oproj(st8):
        op = psC.tile([P, 2, 512], F32, tag="psC", name="op")
        for hp in range(MT):
            first = (hp == 0)
            last = (hp == MT - 1)
            nc.tensor.matmul(op[:, 0, :],
                             o_all[:, hp, st8 * P:(st8 + 1) * P],
                             wo_t[:, hp, 0:512],
                             start=first, stop=last)
            nc.tensor.matmul(op[:, 1, 0:256],
                             o_all[:, hp, st8 * P:(st8 + 1) * P],
                             wo_t[:, hp, 512:768],
                             start=first, stop=last)
        out_sb = outpool.tile([P, E], F32, tag="outsb")
        nc.vector.tensor_tensor(out_sb[:, 0:512], op[:, 0, :], bo_sb[:, 0:512],
                                mybir.AluOpType.add)
        nc.vector.tensor_tensor(out_sb[:, 512:768], op[:, 1, 0:256], bo_sb[:, 512:768],
                                mybir.AluOpType.add)
        nc.sync.dma_start(out[st8 * P:(st8 + 1) * P, :], out_sb[:])

    for st8 in range(4):
        _oproj(st8)
    bc_last = _norm_bc(NOUT - 1)
    _norm_tt(NOUT - 1, bc_last)
    for st8 in range(4, QB // P):
        _oproj(st8)


_NC_CACHE = None


def _get_nc():
    global _NC_CACHE
    if _NC_CACHE is None:
        _NC_CACHE = build_nc()
    return _NC_CACHE


def make_in_maps(query, key_, value, Wq, bq, Wk, bk, Wv, bv, Wo, bo):
    """Host-side sharding + layout prep. Returns list of 8 input dicts."""
    query = np.asarray(query, dtype=np.float32)
    key_ = np.asarray(key_, dtype=np.float32)
    value = np.asarray(value, dtype=np.float32)
    scale = 1.0 / np.sqrt(np.float32(D))

    import ml_dtypes
    BF = ml_dtypes.bfloat16
    # [H, E, D] -> [E, H*D], then head-half slices [E, 384]
    wq_f = np.ascontiguousarray(np.transpose(np.asarray(Wq, np.float32), (1, 0, 2)).reshape(E, E)) * scale
    wk_f = np.ascontiguousarray(np.transpose(np.asarray(Wk, np.float32), (1, 0, 2)).reshape(E, E))
    wv_f = np.ascontiguousarray(np.transpose(np.asarray(Wv, np.float32), (1, 0, 2)).reshape(E, E))
    wo_f = np.ascontiguousarray(np.asarray(Wo, np.float32))

    bq_f = (np.asarray(bq, np.float32).reshape(E) * scale)
    bk_f = np.asarray(bk, np.float32).reshape(E)
    bv_f = np.asarray(bv, np.float32).reshape(E)
    bo_f = np.asarray(bo, np.float32)

    xk_t = [np.ascontiguousarray(key_[b].T).astype(BF) for b in range(B)]
    xv_t = [np.ascontiguousarray(value[b].T).astype(BF) for b in range(B)]

    sel_np = np.zeros((2, P), dtype=np.float16)
    sel_np[0, 0:D] = 1.0
    sel_np[1, D:2 * D] = 1.0

    per_half = {}
    for hh in range(2):
        cols = slice(hh * EH, (hh + 1) * EH)
        # partial bias: bv_half @ Wo_half + bo/2
        bo_eff = bv_f[cols] @ wo_f[cols, :] + 0.5 * bo_f
        per_half[hh] = {
            "wq": np.ascontiguousarray(wq_f[:, cols]).astype(BF),
            "wk": np.ascontiguousarray(wk_f[:, cols]).astype(BF),
            "wv": np.ascontiguousarray(wv_f[:, cols]).astype(BF),
            "wo": np.ascontiguousarray(wo_f[cols, :]).astype(BF),
            "bq": bq_f[cols].reshape(MT, P).T.copy(),
            "bk": bk_f[cols].reshape(MT, P).T.copy(),
            "bo": np.tile(bo_eff.reshape(1, E), (P, 1)).copy(),
        }

    in_maps = []
    for core in range(NCORES):
        b = core // 4
        qh = (core // 2) % 2
        hh = core % 2
        xq_np = np.ascontiguousarray(query[b, qh * QB:(qh + 1) * QB, :].T).astype(BF)
        m = {"xq": xq_np, "xk": xk_t[b], "xv": xv_t[b], "seld": sel_np}
        m.update(per_half[hh])
        in_maps.append(m)
    return in_maps


def assemble(results):
    outp = np.empty((B, S, E), dtype=np.float32)
    for b in range(B):
        for qh in range(2):
            c0 = b * 4 + qh * 2
            outp[b, qh * QB:(qh + 1) * QB, :] = results[c0]["out"] + results[c0 + 1]["out"]
    return outp


def kernel(query, key_, value, Wq, bq, Wk, bk, Wv, bv, Wo, bo):
    nc = _get_nc()
    in_maps = make_in_maps(query, key_, value, Wq, bq, Wk, bk, Wv, bv, Wo, bo)
    res = run_bass_kernel_spmd(nc, in_maps, core_ids=list(range(NCORES)))
    return assemble(res.results)


# revision 111
# speedup vs baseline: 1.0112x; 1.0112x over previous
# Multi-head attention kernel for Trainium2, sharded over 8 NeuronCores.
#
# Sharding: core = (batch b, query-half qh, head-half hh). Each core handles
# 1024 queries of one batch for 6 of the 12 heads, computing k/v projections
# for its 6 heads over the full 2048 keys (2x redundancy across query-halves
# only; cross-core collectives are slower than recompute on this chip). The
# output projection is a partial sum over the core's 384 head-dims; the two
# head-half partners are summed on the host in assemble().
#
# Layout strategy (bf16 matmul operands, fp32 PSUM accumulation/epilogues):
#   - Host pre-transposes activations to [E, S] so the contraction dim (E)
#     lands on SBUF partitions; all matmul operands are bf16 (fp32 matmul
#     lowers to two PE passes).
#   - q^T, k^T computed as [384, S] via lhsT=W chunks; per-partition bias
#     added during the PSUM->SBUF copy.
#   - v computed directly as [keys, 384] with a ones-column per head
#     ([128,16,6,65]) so the PV matmul (M=65) also produces the softmax
#     denominator row for free.
#   - scores^T = [keys, queries] per head: K=64 matmuls; even/odd heads of a
#     pair sit in partition halves 0-63/64-127, emitted adjacently so they
#     land in disjoint PE row groups and stream concurrently (row packing).
#   - exp on ScalarE in [128, 2x512] groups PSUM->SBUF (bf16), streamed
#     straight into the accumulating PV matmul.
#   - softmax normalization runs one iteration behind the attention stream
#     (fp16 K=2 broadcast matmul + DVE multiplies), so the in-order engine
#     queues never stall on its dependency chain.
#   - output projection contracts head pairs as K=128 matmuls; epilogue adds
#     the host-precomputed partial bias (bv_half@Wo_half + bo/2).

import numpy as np
from contextlib import ExitStack

import concourse.bass as bass
import concourse.mybir as mybir
import concourse.tile as tile
from concourse import bacc
from concourse.bass_utils import run_bass_kernel_spmd

F32 = mybir.dt.float32
BF16 = mybir.dt.bfloat16
F16 = mybir.dt.float16
P = 128
E = 768
S = 2048
B = 2
H = 12            # total heads
HC = 6            # heads per core
D = 64
EH = HC * D       # 384 head-dims per core
QB = 1024         # queries per core
NCORES = 8
EC = E // P       # 6 e-chunks (contraction tiles)
KT = S // P       # 16 key tiles
MT = EH // P      # 3 M-tiles for q^T/k^T (384 rows)
NC4 = S // 512    # 4 n-slices of k^T
NOUT = 6          # attention outer iterations: (pair, query-slice)


def build_nc():
    nc = bacc.Bacc("TRN2", debug=False)

    # DRAM I/O (per-core shapes; same NEFF on all 8 cores)
    xq = nc.dram_tensor("xq", (E, QB), BF16, kind="ExternalInput")     # query[b,half].T
    xk = nc.dram_tensor("xk", (E, S), BF16, kind="ExternalInput")      # key[b].T
    xv = nc.dram_tensor("xv", (E, S), BF16, kind="ExternalInput")      # value[b].T
    wq = nc.dram_tensor("wq", (E, EH), BF16, kind="ExternalInput")     # head-half slice, pre-scaled
    wk = nc.dram_tensor("wk", (E, EH), BF16, kind="ExternalInput")
    wv = nc.dram_tensor("wv", (E, EH), BF16, kind="ExternalInput")
    wo = nc.dram_tensor("wo", (EH, E), BF16, kind="ExternalInput")
    bq = nc.dram_tensor("bq", (P, MT), F32, kind="ExternalInput")      # per-partition bias per M-tile
    bk = nc.dram_tensor("bk", (P, MT), F32, kind="ExternalInput")
    bo = nc.dram_tensor("bo", (P, E), F32, kind="ExternalInput")       # bv_half@Wo_half + bo/2
    seld = nc.dram_tensor("seld", (2, P), F16, kind="ExternalInput")   # pair-broadcast selector
    out = nc.dram_tensor("out", (QB, E), F32, kind="ExternalOutput")   # partial (head-half) output

    with tile.TileContext(nc) as tc:
        with ExitStack() as ctx:
            _emit(ctx, tc, nc, xq, xk, xv, wq, wk, wv, wo, bq, bk, bo, seld, out)
    nc.compile()
    return nc


def _emit(ctx, tc, nc, xq, xk, xv, wq, wk, wv, wo, bq, bk, bo, seld, out):
    # ---- pools ----
    persist = ctx.enter_context(tc.tile_pool(name="persist", bufs=1))
    wpool = ctx.enter_context(tc.tile_pool(name="wpool", bufs=2))
    xpool = ctx.enter_context(tc.tile_pool(name="xpool", bufs=2))
    epool = ctx.enter_context(tc.tile_pool(name="epool", bufs=4))
    spool = ctx.enter_context(tc.tile_pool(name="spool", bufs=2))
    outpool = ctx.enter_context(tc.tile_pool(name="outpool", bufs=2))
    # PSUM pools (8 banks total): scores/oproj 4 + PV accumulators 2 +
    # vproj/norm-bc 1 + projection chunks 1
    psA = ctx.enter_context(tc.tile_pool(name="psA", bufs=2, space="PSUM"))
    psB = ctx.enter_context(tc.tile_pool(name="psB", bufs=1, space="PSUM"))
    psC = ctx.enter_context(tc.tile_pool(name="psC", bufs=2, space="PSUM"))
    psD = ctx.enter_context(tc.tile_pool(name="psD", bufs=1, space="PSUM"))

    # ---- persistent SBUF tensors ----
    qT = persist.tile([P, MT, QB], BF16)          # q^T [384, 1024]
    kT = persist.tile([P, MT, S], BF16)           # k^T [384, 2048]
    # key^T/value^T staged per 512-key slice: separate tiles keep dependency
    # tracking slice-granular (one big tile makes every reader wait for the
    # last slice DMA)
    xv_sb = [persist.tile([P, EC, 512], BF16, tag=f"xv{n}", name=f"xv{n}")
             for n in range(NC4)]
    xk_sb = [persist.tile([P, EC, 512], BF16, tag=f"xk{n}", name=f"xk{n}")
             for n in range(NC4)]
    v_sb = persist.tile([P, KT, HC, D + 1], BF16)  # v + ones column per head
    o_all = persist.tile([P, MT, QB], BF16)       # normalized o^T, pair-packed
    bq_sb = persist.tile([P, MT], F32)
    bk_sb = persist.tile([P, MT], F32)
    bo_sb = persist.tile([P, E], F32)
    o_raw = persist.tile([D + 1, HC, 2, 512], F32)  # unnormalized o^T per (head, qs)
    sel2 = persist.tile([2, P], F16)              # pair broadcast selector
    dens = [persist.tile([2, 512], F32, tag=f"dens{it}", name=f"dens{it}")
            for it in range(NOUT)]
    drec = [persist.tile([2, 512], F16, tag=f"drec{it}", name=f"drec{it}")
            for it in range(NOUT)]

    # first-needed DMAs first; constants go on the scalar HWDGE queue
    wq_t = wpool.tile([P, EC, EH], BF16, tag="w18")
    xq_t = xpool.tile([P, EC, QB], BF16, tag="xq")
    for ec in range(EC):
        nc.sync.dma_start(wq_t[:, ec, :], wq[ec * P:(ec + 1) * P, :])
        nc.sync.dma_start(xq_t[:, ec, :], xq[ec * P:(ec + 1) * P, :])
    nc.scalar.dma_start(bq_sb[:], bq[:])
    nc.scalar.dma_start(bk_sb[:], bk[:])
    nc.scalar.dma_start(bo_sb[:], bo[:])
    nc.scalar.dma_start(sel2[:], seld[:])

    # ones columns for denominator (written once; v-proj copies don't touch col D)
    nc.vector.memset(v_sb[:, :, :, D], 1.0)

    # ---- projection chunk emitters ----
    # The projections are consumed M-tile-wise: kT/qT tile mt and v head-pair
    # p feed only attention iterations 2mt/2mt+1 (resp. 2p/2p+1). Each chunk
    # is emitted just-in-time inside the iteration stream, so projection work
    # fills the PE slack under the exp pipeline instead of serializing ahead
    # of it. Chunk PSUM lives in its own 1-bank pool (psD) so the ring never
    # collides with the live PV accumulators.
    def _qproj(mt, n2):
        ps = psD.tile([P, 512], F32, tag="psD", name="qp")
        for ec in range(EC):
            nc.tensor.matmul(ps[:], wq_t[:, ec, mt * P:(mt + 1) * P],
                             xq_t[:, ec, n2 * 512:(n2 + 1) * 512],
                             start=(ec == 0), stop=(ec == EC - 1))
        nc.vector.tensor_scalar_add(qT[:, mt, n2 * 512:(n2 + 1) * 512], ps[:],
                                    bq_sb[:, mt:mt + 1])

    def _kproj(mt, n4):
        ps = psD.tile([P, 512], F32, tag="psD", name="kp")
        for ec in range(EC):
            nc.tensor.matmul(ps[:], wk_t[:, ec, mt * P:(mt + 1) * P],
                             xk_sb[n4][:, ec, :],
                             start=(ec == 0), stop=(ec == EC - 1))
        nc.vector.tensor_scalar_add(kT[:, mt, n4 * 512:(n4 + 1) * 512], ps[:],
                                    bk_sb[:, mt:mt + 1])

    def _vproj(p, kt):
        psv = psB.tile([P, 512], F32, tag="psB", name="psv")
        off = (kt % 4) * P
        for ec in range(EC):
            nc.tensor.matmul(psv[:, 0:P], xv_sb[kt // 4][:, ec, off:off + P],
                             wv_t[:, ec, p * P:(p + 1) * P],
                             start=(ec == 0), stop=(ec == EC - 1))
        # strided copy into per-head slots (leaves ones column intact)
        nc.vector.tensor_copy(v_sb[:, kt, 2 * p:2 * p + 2, 0:D],
                              psv[:, 0:P].rearrange("p (h d) -> p h d", d=D))

    # ---- attention helpers ----
    # Per key tile: both heads' score matmuls are adjacent K=64 ops on
    # disjoint PE row groups (partitions 0-63 / 64-127) -> run concurrently.
    # Softmax normalization is pipelined one iteration behind the attention
    # stream so the in-order PE/DVE queues never stall on its chain.
    def _attn_scores(hp, qs, kt):
        st = psC.tile([P, 2, 512], F32, tag="psC", name="st")
        for i in range(2):
            po = D * i      # partition offset of this head's d-rows
            nc.tensor.matmul(st[:, i, :],
                             kT[po:po + D, hp, kt * P:(kt + 1) * P],
                             qT[po:po + D, hp, qs * 512:(qs + 1) * 512],
                             start=True, stop=True)
        ex = epool.tile([P, 2, 512], BF16, tag="ex")
        nc.scalar.activation(ex[:, :, :], st[:, :, :], mybir.ActivationFunctionType.Exp)
        return ex

    def _attn_pv(o_ps, hp, ex, kt):
        for i in range(2):
            nc.tensor.matmul(o_ps[i][0:D + 1, :],
                             v_sb[:, kt, 2 * hp + i, :],
                             ex[:, i, :],
                             start=(kt == 0), stop=(kt == KT - 1))

    def _drain(o_ps, it):
        # per-head copy releases the PSUM bank (incl. denom row); the
        # denominator pair is DMA-gathered onto partitions 0-1 so ONE
        # partition-parallel DVE reciprocal covers both heads.
        hp, qs = divmod(it, 2)
        for i in range(2):
            nc.vector.tensor_copy(o_raw[:, 2 * hp + i, qs, :], o_ps[i][0:D + 1, :])
            nc.sync.dma_start(dens[it][i:i + 1, :], o_raw[D:D + 1, 2 * hp + i, qs, :])
        with nc.allow_low_precision(reason="fp16 reciprocal feeds fp16 broadcast matmul; den ~1e3, ample range"):
            nc.vector.reciprocal(drec[it][:], dens[it][:])

    # ---- stage k^T/v^T activations and weights; slice 0 ahead of the bulk
    # so the first kproj/vproj chunks are not DMA-gated ----
    wk_t = wpool.tile([P, EC, EH], BF16, tag="w18")
    for ec in range(EC):
        nc.sync.dma_start(wk_t[:, ec, :], wk[ec * P:(ec + 1) * P, :])
    nc.sync.dma_start(xk_sb[0][:], xk[:, 0:512].rearrange("(ec p) s -> p ec s", p=P))
    nc.sync.dma_start(xv_sb[0][:], xv[:, 0:512].rearrange("(ec p) s -> p ec s", p=P))
    wv_t = wpool.tile([P, EC, EH], BF16, tag="wv")
    for ec in range(EC):
        nc.sync.dma_start(wv_t[:, ec, :], wv[ec * P:(ec + 1) * P, :])
    for n4 in range(1, NC4):
        nc.sync.dma_start(xk_sb[n4][:],
                          xk[:, n4 * 512:(n4 + 1) * 512].rearrange("(ec p) s -> p ec s", p=P))
        nc.sync.dma_start(xv_sb[n4][:],
                          xv[:, n4 * 512:(n4 + 1) * 512].rearrange("(ec p) s -> p ec s", p=P))
    _qproj(0, 0)
    _kproj(0, 0)

    def _norm_bc(it):
        # one fp16 K=2 matmul broadcasts both reciprocals of the pair into
        # partition halves of the (idle during attention) psB bank
        bc = psB.tile([P, 512], F32, tag="psB", name="bc")
        nc.tensor.matmul(bc[:], sel2[:], drec[it][:], start=True, stop=True)
        return bc

    def _norm_tt(it, bc):
        hp, qs = divmod(it, 2)
        for i in range(2):
            nc.vector.tensor_tensor(o_all[D * i:D * (i + 1), hp, qs * 512:(qs + 1) * 512],
                                    o_raw[0:D, 2 * hp + i, qs, :],
                                    bc[D * i:D * (i + 1), :], mybir.AluOpType.mult)

    bc_prev = None
    for it in range(NOUT):
        hp, qs = divmod(it, 2)
        o_ps = {}
        for i in range(2):
            o_ps[i] = psA.tile([P, 512], F32, tag="psA", name=f"o_ps{i}")
        for kt in range(KT):
            # scores + exp go first in the slot so the ACT pipeline is fed as
            # early as possible; the projection fillers then run in the PE
            # window while exp computes, and the PVs close the slot.
            ex = _attn_scores(hp, qs, kt)
            if it % 2 == 0:
                # even iterations first consume pair hp: its v chunks emit
                # just-in-time, and kT tile hp's remaining key-slices emit
                # two slots ahead of their first consuming scores. Pairs > 0
                # had their first 8 v chunks emitted in the previous (lighter)
                # odd iteration.
                if hp == 0 or kt >= 8:
                    _vproj(hp, kt)
                if kt in (2, 6, 10):
                    _kproj(hp, kt // 4 + 1)
            else:
                if hp + 1 < MT and kt % 2 == 0:
                    _vproj(hp + 1, kt // 2)
                if kt == 12 and hp + 1 < MT:
                    _kproj(hp + 1, 0)
            if kt == 8 and it + 1 < NOUT:
                nmt, nn2 = divmod(it + 1, 2)
                _qproj(nmt, nn2)
            if it > 0:
                if kt == 10:
                    bc_prev = _norm_bc(it - 1)
                if kt == 14:
                    _norm_tt(it - 1, bc_prev)
            _attn_pv(o_ps, hp, ex, kt)
        _drain(o_ps, it)
    # ---- output projection (partial: this core's 384 head-dims) ----
    # Double-buffered through the psC pool (scores are done with it). The
    # first half (query tiles 0-3) depends only on qs=0 normalizations, which
    # completed before the last attention iteration — emitting it before the
    # final norm keeps the PE busy (and its clock warm) through that chain.
    wo_t = wpool.tile([P, MT, E], BF16, tag="w18")
    nc.sync.dma_start(wo_t[:], wo[:].rearrange("(mt p) m -> p mt m", p=P))

    def _oproj(st8):
        op = psC.tile([P, 2, 512], F32, tag="psC", name="op")
        for hp in range(MT):
            first = (hp == 0)
            last = (hp == MT - 1)
            nc.tensor.matmul(op[:, 0, :],
                             o_all[:, hp, st8 * P:(st8 + 1) * P],
                             wo_t[:, hp, 0:512],
                             start=first, stop=last)
            nc.tensor.matmul(op[:, 1, 0:256],
                             o_all[:, hp, st8 * P:(st8 + 1) * P],
                             wo_t[:, hp, 512:768],
                             start=first, stop=last)
        out_sb = outpool.tile([P, E], F32, tag="outsb")
        nc.vector.tensor_tensor(out_sb[:, 0:512], op[:, 0, :], bo_sb[:, 0:512],
                                mybir.AluOpType.add)
        nc.vector.tensor_tensor(out_sb[:, 512:768], op[:, 1, 0:256], bo_sb[:, 512:768],
                                mybir.AluOpType.add)
        nc.sync.dma_start(out[st8 * P:(st8 + 1) * P, :], out_sb[:])

    for st8 in range(4):
        _oproj(st8)
    bc_last = _norm_bc(NOUT - 1)
    _norm_tt(NOUT - 1, bc_last)
    for st8 in range(4, QB // P):
        _oproj(st8)


_NC_CACHE = None


def _get_nc():
    global _NC_CACHE
    if _NC_CACHE is None:
        _NC_CACHE = build_nc()
    return _NC_CACHE


def make_in_maps(query, key_, value, Wq, bq, Wk, bk, Wv, bv, Wo, bo):
    """Host-side sharding + layout prep. Returns list of 8 input dicts."""
    query = np.asarray(query, dtype=np.float32)
    key_ = np.asarray(key_, dtype=np.float32)
    value = np.asarray(value, dtype=np.float32)
    scale = 1.0 / np.sqrt(np.float32(D))

    import ml_dtypes
    BF = ml_dtypes.bfloat16
    # [H, E, D] -> [E, H*D], then head-half slices [E, 384]
    wq_f = np.ascontiguousarray(np.transpose(np.asarray(Wq, np.float32), (1, 0, 2)).reshape(E, E)) * scale
    wk_f = np.ascontiguousarray(np.transpose(np.asarray(Wk, np.float32), (1, 0, 2)).reshape(E, E))
    wv_f = np.ascontiguousarray(np.transpose(np.asarray(Wv, np.float32), (1, 0, 2)).reshape(E, E))
    wo_f = np.ascontiguousarray(np.asarray(Wo, np.float32))

    bq_f = (np.asarray(bq, np.float32).reshape(E) * scale)
    bk_f = np.asarray(bk, np.float32).reshape(E)
    bv_f = np.asarray(bv, np.float32).reshape(E)
    bo_f = np.asarray(bo, np.float32)

    xk_t = [np.ascontiguousarray(key_[b].T).astype(BF) for b in range(B)]
    xv_t = [np.ascontiguousarray(value[b].T).astype(BF) for b in range(B)]

    sel_np = np.zeros((2, P), dtype=np.float16)
    sel_np[0, 0:D] = 1.0
    sel_np[1, D:2 * D] = 1.0

    per_half = {}
    for hh in range(2):
        cols = slice(hh * EH, (hh + 1) * EH)
        # partial bias: bv_half @ Wo_half + bo/2
        bo_eff = bv_f[cols] @ wo_f[cols, :] + 0.5 * bo_f
        per_half[hh] = {
            "wq": np.ascontiguousarray(wq_f[:, cols]).astype(BF),
            "wk": np.ascontiguousarray(wk_f[:, cols]).astype(BF),
            "wv": np.ascontiguousarray(wv_f[:, cols]).astype(BF),
            "wo": np.ascontiguousarray(wo_f[cols, :]).astype(BF),
            "bq": bq_f[cols].reshape(MT, P).T.copy(),
            "bk": bk_f[cols].reshape(MT, P).T.copy(),
            "bo": np.tile(bo_eff.reshape(1, E), (P, 1)).copy(),
        }

    in_maps = []
    for core in range(NCORES):
        b = core // 4
        qh = (core // 2) % 2
        hh = core % 2
        xq_np = np.ascontiguousarray(query[b, qh * QB:(qh + 1) * QB, :].T).astype(BF)
        m = {"xq": xq_np, "xk": xk_t[b], "xv": xv_t[b], "seld": sel_np}
        m.update(per_half[hh])
        in_maps.append(m)
    return in_maps


def assemble(results):
    outp = np.empty((B, S, E), dtype=np.float32)
    for b in range(B):
        for qh in range(2):
            c0 = b * 4 + qh * 2
            outp[b, qh * QB:(qh + 1) * QB, :] = results[c0]["out"] + results[c0 + 1]["out"]
    return outp


def kernel(query, key_, value, Wq, bq, Wk, bk, Wv, bv, Wo, bo):
    nc = _get_nc()
    in_maps = make_in_maps(query, key_, value, Wq, bq, Wk, bk, Wv, bv, Wo, bo)
    res = run_bass_kernel_spmd(nc, in_maps, core_ids=list(range(NCORES)))
    return assemble(res.results)
